# revision 1
# baseline (speedup 1.0000x reference)
"""Trainium2 Bass kernel for nn_InvLocalPatOrientConvolution.

Computation:
  1. Host: synthesize the 160-channel 5x5x5 conv filter (scaled x32), split
     weights and input into fp8-e4m3 hi/lo pairs (w = w_hi + w_lo exactly at
     fp8 resolution; x = x_hi + x_lo/16), lay out per-core operands.
  2. Device (8 NeuronCores, SPMD): VALID 3D conv as fp8 DoubleRow PE matmuls
     (each instruction contracts 2 k-tiles of 80 rows = 5 y-taps x 16 cin at
     0.5 cycles/col):
       - 25 main DRs per chunk: k-tiles (w_hi, w_lo) x (x_hi, x_hi) via a
         stride-0 broadcast on the rhs plane dim -> full-precision w times x_hi
       - 15 xlo DRs: k-tiles pair adjacent x-planes (i, i+1) of x_lo with
         w_hi/16 weights -> w times x_lo correction
     Compensated fp8 keeps the conv at ~1.3e-3 rel err while halving PE time.
     SO(3) grid pooling (relu-weighted second-moment ratio) stays fp16 on the
     PE; the x32 weight scale is folded into the grid constants.
     Sharding: batch (2) x output-X-slabs (4) -> 8 cores.
  3. Host: gather per-core slabs into the full (2,16,36,36,36) output.
"""

import os
import sys

for _p in ("/root/.axon_site/_ro/trn_rl_repo", "/opt/trn_rl_repo"):
    if os.path.isdir(_p) and _p not in sys.path:
        sys.path.insert(0, _p)

import numpy as np
import ml_dtypes

import concourse.mybir as mybir
from concourse import bacc
from concourse.tile import TileContext
from concourse.bass_utils import run_bass_kernel_spmd

# Problem constants (hardcoded per harness contract)
ORDER = 2
KS = 5            # conv kernel size
CIN = 16
COUT = 16
EPS = 1e-16
S = 10            # wigner rows
B = 2
D_IN = 40         # input spatial
D_OUT = 36        # output spatial
SLAB = 9          # output X planes per core (36/4)
SLAB_IN = SLAB + KS - 1   # 13 input X planes per core
NPL = SLAB_IN + 1         # + zero pad plane for the xlo pair reads
NCORES = 8
YB = 12           # y-block per chunk
NCHUNK = YB * D_OUT       # 432 columns per matmul chunk
WSCALE = 32.0     # filter pre-scale so fp8 hi/lo split keeps precision

F8 = mybir.dt.float8e4
F16 = mybir.dt.float16
F32 = mybir.dt.float32
NPF8 = ml_dtypes.float8_e4m3
DR = mybir.MatmulPerfMode.DoubleRow

_prog_cache = {}


def _conv_chunk(nc, ps, wt8, xh, xl, xr, y0, lo, hi):
    """Emit the 40 DoubleRow matmuls of one conv chunk into psum ps."""
    t = 0
    for i in range(KS):
        for k in range(KS):
            rhs = xh[:, xr + i:xr + i + 1, y0:y0 + YB, k:k + D_OUT] \
                .broadcast_to([80, 2, YB, D_OUT])
            nc.tensor.matmul(ps[:], wt8[:, i * KS + k, :, lo:hi], rhs,
                             start=(t == 0), stop=False, perf_mode=DR)
            t += 1
    q = 0
    for k in range(KS):
        for i0 in (0, 2, 4):
            rhs = xl[:, xr + i0:xr + i0 + 2, y0:y0 + YB, k:k + D_OUT]
            nc.tensor.matmul(ps[:], wt8[:, 25 + q, :, lo:hi], rhs,
                             start=False, stop=(q == 14), perf_mode=DR)
            q += 1


def _build_program():
    """Build the SPMD device program (identical on all 8 cores)."""
    nc = bacc.Bacc("TRN2")

    xh_d = nc.dram_tensor("xh", [80, SLAB_IN, D_OUT, D_IN], F8,
                          kind="ExternalInput")
    xl_d = nc.dram_tensor("xl", [80, SLAB_IN, D_OUT, D_IN], F8,
                          kind="ExternalInput")
    w_d = nc.dram_tensor("w", [80, 40, 2, 160], F8, kind="ExternalInput")
    ga_d = nc.dram_tensor("ga", [128, 4, 108], F16, kind="ExternalInput")
    gb_d = nc.dram_tensor("gb", [32, 108], F16, kind="ExternalInput")
    wnd_d = nc.dram_tensor("wnd", [108, 64], F16, kind="ExternalInput")
    one8_d = nc.dram_tensor("one8", [108, 4, 2, 16], F8, kind="ExternalInput")
    wvec_d = nc.dram_tensor("wvec", [108, 1], F32, kind="ExternalInput")
    bias_d = nc.dram_tensor("bias", [16, 1], F32, kind="ExternalInput")
    y_d = nc.dram_tensor("y", [16, SLAB, D_OUT, D_OUT], F32,
                         kind="ExternalOutput")

    chunks = [(xr, cy) for xr in range(SLAB) for cy in range(3)]

    with TileContext(nc) as tc:
        with tc.tile_pool(name="const", bufs=1) as cpool, \
             tc.tile_pool(name="work", bufs=4) as wpool, \
             tc.tile_pool(name="out3", bufs=2) as opool, \
             tc.tile_pool(name="casb", bufs=6) as capool, \
             tc.tile_pool(name="rrel", bufs=10) as rpool, \
             tc.tile_pool(name="conv_ps", bufs=2, space="PSUM") as conv_pool, \
             tc.tile_pool(name="convb_ps", bufs=1, space="PSUM") as convb_pool, \
             tc.tile_pool(name="a_ps", bufs=2, space="PSUM") as a_pool, \
             tc.tile_pool(name="nd_ps", bufs=2, space="PSUM") as nd_pool, \
             tc.tile_pool(name="den_ps", bufs=1, space="PSUM") as den_pool:

            # ---- resident constants. DMA order: what chunk 0 needs first.
            xh = cpool.tile([80, NPL, D_OUT, D_IN], F8, tag="xh")
            xl = cpool.tile([80, NPL, D_OUT, D_IN], F8, tag="xl")
            wt8 = cpool.tile([80, 40, 2, 160], F8, tag="wt8")
            dma_engs = [nc.sync, nc.scalar, nc.gpsimd]
            def _flat(ap):
                return ap.rearrange("p a b c -> p (a b c)")
            for p in range(KS):
                dma_engs[p % 3].dma_start(
                    out=xh[:, p].rearrange("p a b -> p (a b)"),
                    in_=xh_d[:, p].rearrange("p a b -> p (a b)"))
            def _flatw(ap):
                return ap.rearrange("p a b c -> p (a b c)")
            nc.sync.dma_start(out=_flatw(wt8[:, 0:4]), in_=_flatw(w_d[:, 0:4]))
            nc.scalar.dma_start(out=_flatw(wt8[:, 4:12]),
                                in_=_flatw(w_d[:, 4:12]))
            nc.gpsimd.dma_start(out=_flatw(wt8[:, 12:25]),
                                in_=_flatw(w_d[:, 12:25]))
            for p in range(KS + 1):
                dma_engs[p % 3].dma_start(
                    out=xl[:, p].rearrange("p a b -> p (a b)"),
                    in_=xl_d[:, p].rearrange("p a b -> p (a b)"))
            nc.scalar.dma_start(out=_flatw(wt8[:, 25:40]),
                                in_=_flatw(w_d[:, 25:40]))
            nc.gpsimd.memset(xl[:, NPL - 1], 0.0)
            gat = cpool.tile([128, 4, 108], F16)
            gbt = cpool.tile([32, 108], F16)
            wndt = cpool.tile([108, 64], F16)
            one8t = cpool.tile([108, 4, 2, 16], F8)
            wvect = cpool.tile([108, 1], F32)
            biast = cpool.tile([16, 1], F32)
            nc.sync.dma_start(out=gat[:], in_=ga_d[:])
            nc.sync.dma_start(out=gbt[:], in_=gb_d[:])
            nc.sync.dma_start(out=wndt[:], in_=wnd_d[:])
            nc.sync.dma_start(out=one8t[:], in_=one8_d[:])
            nc.sync.dma_start(out=wvect[:], in_=wvec_d[:])
            nc.sync.dma_start(out=biast[:], in_=bias_d[:])
            for p in range(KS, SLAB_IN):
                dma_engs[p % 3].dma_start(
                    out=xh[:, p].rearrange("p a b -> p (a b)"),
                    in_=xh_d[:, p].rearrange("p a b -> p (a b)"))
            for p in range(KS + 1, SLAB_IN):
                dma_engs[p % 3].dma_start(
                    out=xl[:, p].rearrange("p a b -> p (a b)"),
                    in_=xl_d[:, p].rearrange("p a b -> p (a b)"))

            pending = None
            o3 = {"t": None}
            for (xr, cy) in chunks:
                y0 = cy * YB
                # ---- conv A (128 ch) + B (32 ch), compensated fp8 DR
                cps = conv_pool.tile([128, NCHUNK], F32, tag="cps")
                _conv_chunk(nc, cps, wt8, xh, xl, xr, y0, 0, 128)
                ca = capool.tile([128, NCHUNK], F16, tag="ca")
                nc.scalar.copy(ca[:], cps[:])

                cbps = convb_pool.tile([32, NCHUNK], F32, tag="cbps")
                _conv_chunk(nc, cbps, wt8, xh, xl, xr, y0, 128, 160)
                cb = capool.tile([32, NCHUNK], F16, tag="cb")
                nc.vector.tensor_copy(cb[:], cbps[:])

                # ---- so3 grid + relu/square (moments lag one chunk)
                rrels, r2s = [], []
                for mt in range(4):
                    aps = a_pool.tile([108, NCHUNK], F32, tag="aps")
                    last = (mt == 3)
                    nc.tensor.matmul(aps[:], gat[:, mt, :], ca[:],
                                     start=True, stop=not last)
                    if last:
                        nc.tensor.matmul(aps[:], gbt[:], cb[:],
                                         start=False, stop=True)
                    wrel = rpool.tile([108, NCHUNK], F16, tag="rrel")
                    nc.scalar.activation(wrel[:], aps[:],
                                         mybir.ActivationFunctionType.Relu,
                                         scale=wvect[:, 0:1])
                    w8 = rpool.tile([108, 2, NCHUNK], F8, tag="w8")
                    nc.scalar.activation(w8[:, 0, :], aps[:],
                                         mybir.ActivationFunctionType.Relu,
                                         scale=wvect[:, 0:1])
                    nc.vector.tensor_sub(w8[:, 1, :], wrel[:], w8[:, 0, :])
                    r2 = rpool.tile([108, NCHUNK], F16, tag="r2")
                    nc.vector.tensor_mul(r2[:], wrel[:], wrel[:])
                    rrels.append(w8)
                    r2s.append(r2)
                nd_ps = nd_pool.tile([16, NCHUNK], F32, tag="nd")
                den_ps = den_pool.tile([16, NCHUNK], F32, tag="dn")
                if pending is not None:
                    _emit_moments(nc, wndt, one8t, biast, wpool, opool, o3,
                                  y_d, pending)
                pending = (nd_ps, den_ps, rrels, r2s, xr, y0)
            if pending is not None:
                _emit_moments(nc, wndt, one8t, biast, wpool, opool, o3, y_d,
                              pending)

    nc.finalize()
    return nc


def _emit_moments(nc, wndt, one8t, biast, wpool, opool, o3, y_d, st):
    """Emit the 8 col-group-packed moment matmuls + finalize + store for a
    chunk whose grid stage (a/relu/square) was already emitted."""
    nd_ps, den_ps, rrels, r2s, xr, y0 = st
    for mt in range(4):
        wnd_g = wndt[:, mt * 16:(mt + 1) * 16]
        nc.tensor.matmul(nd_ps[:], wnd_g, r2s[mt][:],
                         start=(mt == 0), stop=(mt == 3))
        nc.tensor.matmul(den_ps[:], one8t[:, mt, :, :], rrels[mt][:],
                         start=(mt == 0), stop=(mt == 3), perf_mode=DR)

    num_sb = wpool.tile([16, NCHUNK], F32, tag="num_sb")
    nc.scalar.copy(num_sb[:], nd_ps[:])
    den_sb = wpool.tile([16, NCHUNK], F32, tag="den_sb")
    nc.scalar.activation(den_sb[:], den_ps[:],
         mybir.ActivationFunctionType.Copy,
         bias=EPS)
    recip = wpool.tile([16, NCHUNK], F32, tag="recip")
    nc.vector.reciprocal(recip[:], den_sb[:])
    out_sb = wpool.tile([16, NCHUNK], F32, tag="out_sb")
    nc.vector.tensor_mul(out_sb[:], num_sb[:], recip[:])
    nc.vector.tensor_scalar_add(out_sb[:], out_sb[:], biast[:, 0:1])
    dst = y_d[:, xr].rearrange("p a b -> p (a b)")[
        :, y0 * D_OUT:(y0 + YB) * D_OUT]
    nc.sync.dma_start(out=dst, in_=out_sb[:])


def _synthesize_filter(weight, zeroweight, basis_functions, wig_w, wig_b):
    """Replicate the reference's kernel synthesis in fp32 numpy.

    Returns kern6[l, e, d, i, j, k] of shape (10, 16, 16, 5, 5, 5)."""
    zero_ext = np.concatenate(
        [zeroweight[None, None],
         np.zeros((ORDER ** 2 - 1, 1, CIN, COUT), weight.dtype)], axis=0)
    wfull = np.concatenate([zero_ext, weight], axis=1)       # (4, 10, 16, 16)
    wg = wfull[wig_w]                                        # (10, 10, 16, 16)
    bg = basis_functions[wig_b]                              # (10, 10, 5, 5, 5)
    kern6 = np.einsum("lred,lrijk->ledijk", wg, bg)          # (10,16,16,5,5,5)
    return np.ascontiguousarray(kern6.astype(np.float32))


def _host_prep(x, weight, zeroweight, bias, so3basisgrid, w_i,
               basis_functions, wig_w, wig_b):
    kern6 = _synthesize_filter(weight, zeroweight, basis_functions, wig_w, wig_b)

    # conv weights W[tap, (j,d), ch] scaled x32, cols e-major (A = 0..127)
    w6 = np.ascontiguousarray(
        kern6.transpose(3, 5, 4, 2, 1, 0).reshape(25, 80, 160)
    ).astype(np.float32) * WSCALE
    w_hi = w6.astype(NPF8)
    w_lo = (w6 - w_hi.astype(np.float32)).astype(NPF8)
    w_xlo = (w6 / 16).astype(NPF8)
    w_arr = np.zeros((80, 40, 2, 160), NPF8)
    for tp in range(25):
        w_arr[:, tp, 0, :] = w_hi[tp]
        w_arr[:, tp, 1, :] = w_lo[tp]
    q = 0
    for k in range(KS):
        for i0 in (0, 2, 4):
            w_arr[:, 25 + q, 0, :] = w_xlo[i0 * KS + k]
            if i0 + 1 < KS:
                w_arr[:, 25 + q, 1, :] = w_xlo[(i0 + 1) * KS + k]
            q += 1

    g2 = so3basisgrid.reshape(27, S).astype(np.float32) / WSCALE
    g2t16 = g2.T.astype(np.float16)                          # [l, mln]

    # A-tile so3 lhsT: ga[p, mt, el2*27+mln]; p = e*10+l (p < 128)
    ga = np.zeros((128, 4, 108), np.float16)
    for mt in range(4):
        for el2 in range(4):
            e = 4 * mt + el2
            for l in range(S):
                p = e * S + l
                if p < 128:
                    ga[p, mt, el2 * 27:(el2 + 1) * 27] = g2t16[l]
    # B-tile so3 lhsT (e 12..15 remainder channels), single slot:
    # B row r: r=0,1 -> (e12, l8+r); r=2+10*m+l -> (e13+m, l)
    gb = np.zeros((32, 108), np.float16)
    for r in range(32):
        if r < 2:
            e, l = 12, 8 + r
        else:
            e, l = 13 + (r - 2) // S, (r - 2) % S
        el2 = e - 12
        gb[r, el2 * 27:(el2 + 1) * 27] = g2t16[l]

    # weighted-moment lhsT: wnd[(el2*27+mln), mt*16+e], e = 4mt+el2
    w_flat = np.asarray(w_i, np.float32)[(np.arange(27) // 3) % 3]
    wnd = np.zeros((108, 4, 16), np.float16)
    one8 = np.zeros((108, 4, 2, 16), NPF8)
    for mt in range(4):
        for el2 in range(4):
            e = 4 * mt + el2
            wnd[el2 * 27:(el2 + 1) * 27, mt, e] = \
                (1.0 / w_flat).astype(np.float16)
            one8[el2 * 27:(el2 + 1) * 27, mt, :, e] = 1.0
    wnd = wnd.reshape(108, 64)
    wvec = np.tile(w_flat, 4).reshape(108, 1).astype(np.float32)

    bias_arr = np.asarray(bias, np.float32).reshape(16, 1)

    x = np.asarray(x, np.float32)
    xh_all = x.astype(NPF8)
    xl_all = ((x - xh_all.astype(np.float32)) * 16).astype(NPF8)

    in_maps = []
    for c in range(NCORES):
        b, qq = divmod(c, 4)
        xh_arr = np.empty((80, SLAB_IN, D_OUT, D_IN), NPF8)
        xl_arr = np.empty((80, SLAB_IN, D_OUT, D_IN), NPF8)
        for j in range(KS):
            for d in range(CIN):
                sl = slice(qq * SLAB, qq * SLAB + SLAB_IN)
                xh_arr[j * CIN + d] = xh_all[b, d, sl, j:j + D_OUT, :]
                xl_arr[j * CIN + d] = xl_all[b, d, sl, j:j + D_OUT, :]
        in_maps.append({
            "xh": np.ascontiguousarray(xh_arr),
            "xl": np.ascontiguousarray(xl_arr),
            "w": w_arr,
            "ga": np.ascontiguousarray(ga),
            "gb": np.ascontiguousarray(gb),
            "wnd": np.ascontiguousarray(wnd),
            "one8": np.ascontiguousarray(one8),
            "wvec": np.ascontiguousarray(wvec),
            "bias": bias_arr,
        })
    return in_maps


def _run(inputs, trace=False, **run_kwargs):
    inputs = {k: np.asarray(v) for k, v in inputs.items()}
    in_maps = _host_prep(**inputs)
    if "nc" not in _prog_cache:
        _prog_cache["nc"] = _build_program()
    nc = _prog_cache["nc"]
    try:
        res = run_bass_kernel_spmd(nc, in_maps, core_ids=list(range(NCORES)),
                                   trace=trace, **run_kwargs)
    except ModuleNotFoundError as e:
        if "axon_hooks" not in str(e):
            raise
        # Tracing requested (e.g. BASS_TRACE=1) but this axon client has no
        # NTFF profile hook - rerun with tracing disabled.
        os.environ["BASS_NEVER_TRACE"] = "1"
        res = run_bass_kernel_spmd(nc, in_maps, core_ids=list(range(NCORES)),
                                   trace=False, **run_kwargs)
    out = np.empty((B, COUT, D_OUT, D_OUT, D_OUT), np.float32)
    for c in range(NCORES):
        b, qq = divmod(c, 4)
        out[b, :, qq * SLAB:(qq + 1) * SLAB] = res.results[c]["y"]
    return out, res


def kernel(**inputs):
    out, _ = _run(inputs)
    return out



# revision 8
# speedup vs baseline: 1.2456x; 1.2456x over previous
"""Trainium2 Bass kernel for nn_InvLocalPatOrientConvolution.

Computation:
  1. Host: synthesize the 160-channel 5x5x5 conv filter (scaled x32), split
     weights and input into fp8-e4m3 hi/lo pairs (w = w_hi + w_lo exactly at
     fp8 resolution; x = x_hi + x_lo/16), lay out per-core operands.
  2. Device (8 NeuronCores, SPMD): VALID 3D conv as fp8 DoubleRow PE matmuls.
     The conv contraction (125 taps x 16 cin = 2000 rows) is packed onto the
     full 128 SBUF partitions: the host materializes (j,k)-shifted copies of
     x so that rows = 8 (j,k)-taps x 16 cin per tile (3 tiles = taps 0..23),
     plus an 80-row "runt" tensor holding tap 24 = (j=4,k=4) with the i-shift
     baked in (rows = 5 i-planes x 16 cin).  Per chunk and per output-channel
     group this gives:
       - 16 main DRs: (w_hi, w_lo) x broadcast x_hi   (full-precision w)
       - ~9 xlo DRs:  w/16 x x_lo pairs (tile pairs / plane pairs / runt)
     i.e. ~50 matmuls/chunk vs 80 for an 80-row packing.
     SO(3) grid pooling (relu-weighted second-moment ratio) stays fp16 on the
     PE; the x32 weight scale is folded into the grid constants.
     Sharding: batch (2) x output-X-slabs (4) -> 8 cores.
     x copies stream through a 7-plane SBUF ring (window of 5 + prefetch).
  3. Host: gather per-core slabs into the full (2,16,36,36,36) output.
"""

import os
import sys

for _p in ("/root/.axon_site/_ro/trn_rl_repo", "/opt/trn_rl_repo"):
    if os.path.isdir(_p) and _p not in sys.path:
        sys.path.insert(0, _p)

import numpy as np
import ml_dtypes

import concourse.mybir as mybir
from concourse import bacc
from concourse.tile import TileContext
from concourse.bass_utils import run_bass_kernel_spmd

# Problem constants (hardcoded per harness contract)
ORDER = 2
KS = 5            # conv kernel size
CIN = 16
COUT = 16
EPS = 1e-16
S = 10            # wigner rows
B = 2
D_IN = 40         # input spatial
D_OUT = 36        # output spatial
SLAB = 9          # output X planes per core (36/4)
SLAB_IN = SLAB + KS - 1   # 13 input X planes per core
NCORES = 8
YB = 12           # y-block per chunk
NCHUNK = YB * D_OUT       # 432 columns per matmul chunk
WSCALE = 32.0     # filter pre-scale so fp8 hi/lo split keeps precision

# weight slot layout in wt tensor: [128, NWSLOT, 2, 160]
#   slots 0..14   main (i*3+t): (w_hi, w_lo) for taps t*8..t*8+7 at plane i
#   slots 15..19  xlo tile-pair (i): (w16[i,t0], w16[i,t1])
#   slots 20..21  xlo t2 plane-pair (i=0,2): (w16[i,t2], w16[i+1,t2])
#   slots 22..26  xlo t2 single (i=0..4): (w16[i,t2], 0)
NW_MAIN = 0
NW_XPAIR = 15
NW_T2PAIR = 20
NW_T2SINGLE = 22
NWSLOT = 27

F8 = mybir.dt.float8e4
F16 = mybir.dt.float16
F32 = mybir.dt.float32
NPF8 = ml_dtypes.float8_e4m3
DR = mybir.MatmulPerfMode.DoubleRow

_prog_cache = {}


def _conv_chunk(nc, ps, wt, wr, xm, xlm, xrh, xrl, xr, y0, lo, hi):
    """Emit the ~25 DoubleRow matmuls of one conv chunk into psum ps."""
    first = True

    def mm(lhsT, rhs, stop=False):
        nonlocal first
        nc.tensor.matmul(ps[:], lhsT, rhs, start=first, stop=stop,
                         perf_mode=DR)
        first = False

    # main: (w_hi, w_lo) x broadcast x_hi
    for i in range(KS):
        s = xr + i
        for t in range(3):
            rhs = xm[:, t, s:s + 1, y0:y0 + YB, :] \
                .broadcast_to([128, 2, YB, D_OUT])
            mm(wt[:, NW_MAIN + i * 3 + t, :, lo:hi], rhs)
    mm(wr[:, 0, :, lo:hi],
       xrh[:, xr:xr + 1, y0:y0 + YB, :].broadcast_to([80, 2, YB, D_OUT]))

    # xlo: w/16 x x_lo, pairing (t0,t1) tiles per plane
    for i in range(KS):
        mm(wt[:, NW_XPAIR + i, :, lo:hi], xlm[:, 0:2, xr + i, y0:y0 + YB, :])
    # xlo t2 tile: plane pairs (0,1), (2,3); plane 4 solo
    for i in (0, 2):
        s = xr + i
        mm(wt[:, NW_T2PAIR + i // 2, :, lo:hi],
           xlm[:, 2, s:s + 2, y0:y0 + YB, :])
    s = xr + 4
    mm(wt[:, NW_T2SINGLE + 4, :, lo:hi],
       xlm[:, 2, s:s + 1, y0:y0 + YB, :].broadcast_to([128, 2, YB, D_OUT]))
    # xlo runt tap
    mm(wr[:, 1, :, lo:hi],
       xrl[:, xr:xr + 1, y0:y0 + YB, :].broadcast_to([80, 2, YB, D_OUT]),
       stop=True)


def _build_program():
    """Build the SPMD device program (identical on all 8 cores)."""
    nc = bacc.Bacc("TRN2")

    xm_d = nc.dram_tensor("xm", [128, 3, SLAB_IN, D_OUT, D_OUT], F8,
                          kind="ExternalInput")
    xlm_d = nc.dram_tensor("xlm", [128, 3, SLAB_IN, D_OUT, D_OUT], F8,
                           kind="ExternalInput")
    xrh_d = nc.dram_tensor("xrh", [80, SLAB, D_OUT, D_OUT], F8,
                           kind="ExternalInput")
    xrl_d = nc.dram_tensor("xrl", [80, SLAB, D_OUT, D_OUT], F8,
                           kind="ExternalInput")
    wt_d = nc.dram_tensor("wt", [128, NWSLOT, 2, 160], F8,
                          kind="ExternalInput")
    wr_d = nc.dram_tensor("wr", [80, 2, 2, 160], F8, kind="ExternalInput")
    ga_d = nc.dram_tensor("ga", [128, 4, 108], F16, kind="ExternalInput")
    gb_d = nc.dram_tensor("gb", [32, 108], F16, kind="ExternalInput")
    wnd_d = nc.dram_tensor("wnd", [108, 64], F16, kind="ExternalInput")
    one8_d = nc.dram_tensor("one8", [108, 4, 2, 16], F8, kind="ExternalInput")
    wvec_d = nc.dram_tensor("wvec", [108, 1], F32, kind="ExternalInput")
    bias_d = nc.dram_tensor("bias", [16, 1], F32, kind="ExternalInput")
    y_d = nc.dram_tensor("y", [16, SLAB, D_OUT, D_OUT], F32,
                         kind="ExternalOutput")

    chunks = [(xr, cy) for xr in range(SLAB) for cy in range(3)]

    with TileContext(nc) as tc:
        with tc.tile_pool(name="const", bufs=1) as cpool, \
             tc.tile_pool(name="work", bufs=4) as wpool, \
             tc.tile_pool(name="casb", bufs=6) as capool, \
             tc.tile_pool(name="rrel", bufs=10) as rpool, \
             tc.tile_pool(name="conv_ps", bufs=2, space="PSUM") as conv_pool, \
             tc.tile_pool(name="convb_ps", bufs=1, space="PSUM") as convb_pool, \
             tc.tile_pool(name="a_ps", bufs=2, space="PSUM") as a_pool, \
             tc.tile_pool(name="nd_ps", bufs=2, space="PSUM") as nd_pool, \
             tc.tile_pool(name="den_ps", bufs=1, space="PSUM") as den_pool:

            # ---- resident constants + x tap-copies (all planes SBUF-resident)
            xm = cpool.tile([128, 3, SLAB_IN, D_OUT, D_OUT], F8, tag="xm")
            xlm = cpool.tile([128, 3, SLAB_IN, D_OUT, D_OUT], F8, tag="xlm")
            xrh = cpool.tile([80, SLAB, D_OUT, D_OUT], F8, tag="xrh")
            xrl = cpool.tile([80, SLAB, D_OUT, D_OUT], F8, tag="xrl")
            wt = cpool.tile([128, NWSLOT, 2, 160], F8, tag="wt")
            wr = cpool.tile([80, 2, 2, 160], F8, tag="wr")
            dma_engs = [nc.sync, nc.scalar, nc.gpsimd]

            def _ld_planes(xt, xt_d, p0, p1, qoff=0):
                """DMA planes [p0,p1) of all 3 tiles, one DMA per tile."""
                for t in range(3):
                    dma_engs[(t + qoff) % 3].dma_start(
                        out=xt[:, t, p0:p1].rearrange("p a b c -> p (a b c)"),
                        in_=xt_d[:, t, p0:p1].rearrange("p a b c -> p (a b c)"))

            def _flat(ap):
                return ap.rearrange("p a b c -> p (a b c)")

            # chunk-0 critical data first: main planes 0-4 + weights + runts
            _ld_planes(xm, xm_d, 0, KS)
            nc.sync.dma_start(out=_flat(wt[:, 0:15]), in_=_flat(wt_d[:, 0:15]))
            nc.scalar.dma_start(out=_flat(wt[:, 15:NWSLOT]),
                                in_=_flat(wt_d[:, 15:NWSLOT]))
            nc.gpsimd.dma_start(out=_flat(wr[:]), in_=_flat(wr_d[:]))
            _ld_planes(xlm, xlm_d, 0, KS, qoff=1)
            nc.sync.dma_start(
                out=xrh.rearrange("p a b c -> p (a b c)"),
                in_=xrh_d.rearrange("p a b c -> p (a b c)"))
            nc.scalar.dma_start(
                out=xrl.rearrange("p a b c -> p (a b c)"),
                in_=xrl_d.rearrange("p a b c -> p (a b c)"))
            gat = cpool.tile([128, 4, 108], F16)
            gbt = cpool.tile([32, 108], F16)
            wndt = cpool.tile([108, 64], F16)
            one8t = cpool.tile([108, 4, 2, 16], F8)
            wvect = cpool.tile([108, 1], F32)
            biast = cpool.tile([16, 1], F32)
            nc.sync.dma_start(out=gat[:], in_=ga_d[:])
            nc.sync.dma_start(out=gbt[:], in_=gb_d[:])
            nc.sync.dma_start(out=wndt[:], in_=wnd_d[:])
            nc.sync.dma_start(out=one8t[:], in_=one8_d[:])
            nc.sync.dma_start(out=wvect[:], in_=wvec_d[:])
            nc.sync.dma_start(out=biast[:], in_=bias_d[:])
            # remaining planes (bulk, overlap with early chunks)
            _ld_planes(xm, xm_d, KS, SLAB_IN)
            _ld_planes(xlm, xlm_d, KS, SLAB_IN, qoff=1)

            pending = None
            for (xr, cy) in chunks:
                y0 = cy * YB
                # ---- conv A (128 ch) + B (32 ch), compensated fp8 DR
                cps = conv_pool.tile([128, NCHUNK], F32, tag="cps")
                _conv_chunk(nc, cps, wt, wr, xm, xlm, xrh, xrl, xr, y0,
                            0, 128)
                ca = capool.tile([128, NCHUNK], F16, tag="ca")
                nc.scalar.copy(ca[:], cps[:])

                cbps = convb_pool.tile([32, NCHUNK], F32, tag="cbps")
                _conv_chunk(nc, cbps, wt, wr, xm, xlm, xrh, xrl, xr, y0,
                            128, 160)
                cb = capool.tile([32, NCHUNK], F16, tag="cb")
                nc.vector.tensor_copy(cb[:], cbps[:])

                # ---- so3 grid + relu/square (moments lag one chunk)
                rrels, r2s = [], []
                for mt in range(4):
                    aps = a_pool.tile([108, NCHUNK], F32, tag="aps")
                    last = (mt == 3)
                    nc.tensor.matmul(aps[:], gat[:, mt, :], ca[:],
                                     start=True, stop=not last)
                    if last:
                        nc.tensor.matmul(aps[:], gbt[:], cb[:],
                                         start=False, stop=True)
                    wrel = rpool.tile([108, NCHUNK], F16, tag="rrel")
                    nc.scalar.activation(wrel[:], aps[:],
                                         mybir.ActivationFunctionType.Relu,
                                         scale=wvect[:, 0:1])
                    w8 = rpool.tile([108, 2, NCHUNK], F8, tag="w8")
                    nc.scalar.activation(w8[:, 0, :], aps[:],
                                         mybir.ActivationFunctionType.Relu,
                                         scale=wvect[:, 0:1])
                    nc.vector.tensor_sub(w8[:, 1, :], wrel[:], w8[:, 0, :])
                    r2 = rpool.tile([108, NCHUNK], F16, tag="r2")
                    nc.vector.tensor_mul(r2[:], wrel[:], wrel[:])
                    rrels.append(w8)
                    r2s.append(r2)
                nd_ps = nd_pool.tile([16, NCHUNK], F32, tag="nd")
                den_ps = den_pool.tile([16, NCHUNK], F32, tag="dn")
                if pending is not None:
                    _emit_moments(nc, wndt, one8t, biast, wpool, y_d, pending)
                pending = (nd_ps, den_ps, rrels, r2s, xr, y0)
            if pending is not None:
                _emit_moments(nc, wndt, one8t, biast, wpool, y_d, pending)

    nc.finalize()
    return nc


def _emit_moments(nc, wndt, one8t, biast, wpool, y_d, st):
    """Emit the 8 moment matmuls + finalize + store for a chunk whose grid
    stage (a/relu/square) was already emitted."""
    nd_ps, den_ps, rrels, r2s, xr, y0 = st
    for mt in range(4):
        wnd_g = wndt[:, mt * 16:(mt + 1) * 16]
        nc.tensor.matmul(nd_ps[:], wnd_g, r2s[mt][:],
                         start=(mt == 0), stop=(mt == 3))
        nc.tensor.matmul(den_ps[:], one8t[:, mt, :, :], rrels[mt][:],
                         start=(mt == 0), stop=(mt == 3), perf_mode=DR)

    num_sb = wpool.tile([16, NCHUNK], F32, tag="num_sb")
    nc.scalar.copy(num_sb[:], nd_ps[:])
    den_sb = wpool.tile([16, NCHUNK], F32, tag="den_sb")
    nc.scalar.activation(den_sb[:], den_ps[:],
         mybir.ActivationFunctionType.Copy,
         bias=EPS)
    recip = wpool.tile([16, NCHUNK], F32, tag="recip")
    nc.vector.reciprocal(recip[:], den_sb[:])
    out_sb = wpool.tile([16, NCHUNK], F32, tag="out_sb")
    nc.vector.tensor_mul(out_sb[:], num_sb[:], recip[:])
    nc.vector.tensor_scalar_add(out_sb[:], out_sb[:], biast[:, 0:1])
    dst = y_d[:, xr].rearrange("p a b -> p (a b)")[
        :, y0 * D_OUT:(y0 + YB) * D_OUT]
    nc.sync.dma_start(out=dst, in_=out_sb[:])


def _synthesize_filter(weight, zeroweight, basis_functions, wig_w, wig_b):
    """Replicate the reference's kernel synthesis in fp32 numpy.

    Returns kern6[l, e, d, i, j, k] of shape (10, 16, 16, 5, 5, 5)."""
    zero_ext = np.concatenate(
        [zeroweight[None, None],
         np.zeros((ORDER ** 2 - 1, 1, CIN, COUT), weight.dtype)], axis=0)
    wfull = np.concatenate([zero_ext, weight], axis=1)       # (4, 10, 16, 16)
    wg = wfull[wig_w]                                        # (10, 10, 16, 16)
    bg = basis_functions[wig_b]                              # (10, 10, 5, 5, 5)
    kern6 = np.einsum("lred,lrijk->ledijk", wg, bg)          # (10,16,16,5,5,5)
    return np.ascontiguousarray(kern6.astype(np.float32))


def _host_prep(x, weight, zeroweight, bias, so3basisgrid, w_i,
               basis_functions, wig_w, wig_b):
    kern6 = _synthesize_filter(weight, zeroweight, basis_functions, wig_w, wig_b)

    # conv weights w6[(i,j,k), (jrow? no) ...]: scaled x32, cols (e*10+l)
    # w6[tap, cin, col] with tap = i*25 + j*5 + k
    w6 = np.ascontiguousarray(
        kern6.transpose(3, 4, 5, 2, 1, 0).reshape(125, CIN, 160)
    ).astype(np.float32) * WSCALE
    w_hi = w6.astype(NPF8)
    w_lo = (w6 - w_hi.astype(np.float32)).astype(NPF8)
    w_x16 = (w6 / 16).astype(NPF8)

    # wt[row=(jkl*16+cin), slot, 2, 160]
    wt_arr = np.zeros((128, NWSLOT, 2, 160), NPF8)
    for i in range(KS):
        for t in range(3):
            for jkl in range(8):
                jk = t * 8 + jkl
                j, k = jk // KS, jk % KS
                tap = i * 25 + j * 5 + k
                r0 = jkl * CIN
                wt_arr[r0:r0 + CIN, NW_MAIN + i * 3 + t, 0, :] = w_hi[tap]
                wt_arr[r0:r0 + CIN, NW_MAIN + i * 3 + t, 1, :] = w_lo[tap]
        # xlo tile pair (t0, t1) at plane i
        for sl, t in ((0, 0), (1, 1)):
            for jkl in range(8):
                jk = t * 8 + jkl
                j, k = jk // KS, jk % KS
                tap = i * 25 + j * 5 + k
                r0 = jkl * CIN
                wt_arr[r0:r0 + CIN, NW_XPAIR + i, sl, :] = w_x16[tap]
    # xlo t2 (taps 16..23) plane pairs and singles
    def _t2_block(dst_slot, sl, i):
        for jkl in range(8):
            jk = 16 + jkl
            j, k = jk // KS, jk % KS
            tap = i * 25 + j * 5 + k
            r0 = jkl * CIN
            wt_arr[r0:r0 + CIN, dst_slot, sl, :] = w_x16[tap]
    for pi, i in enumerate((0, 2)):
        _t2_block(NW_T2PAIR + pi, 0, i)
        _t2_block(NW_T2PAIR + pi, 1, i + 1)
    for i in range(KS):
        _t2_block(NW_T2SINGLE + i, 0, i)

    # runt weights: tap (i, 4, 4), rows (i*16+cin)
    wr_arr = np.zeros((80, 2, 2, 160), NPF8)
    for i in range(KS):
        tap = i * 25 + 4 * 5 + 4
        r0 = i * CIN
        wr_arr[r0:r0 + CIN, 0, 0, :] = w_hi[tap]
        wr_arr[r0:r0 + CIN, 0, 1, :] = w_lo[tap]
        wr_arr[r0:r0 + CIN, 1, 0, :] = w_x16[tap]

    g2 = so3basisgrid.reshape(27, S).astype(np.float32) / WSCALE
    g2t16 = g2.T.astype(np.float16)                          # [l, mln]

    # A-tile so3 lhsT: ga[p, mt, el2*27+mln]; p = e*10+l (p < 128)
    ga = np.zeros((128, 4, 108), np.float16)
    for mt in range(4):
        for el2 in range(4):
            e = 4 * mt + el2
            for l in range(S):
                p = e * S + l
                if p < 128:
                    ga[p, mt, el2 * 27:(el2 + 1) * 27] = g2t16[l]
    # B-tile so3 lhsT (e 12..15 remainder channels), single slot:
    # B row r: r=0,1 -> (e12, l8+r); r=2+10*m+l -> (e13+m, l)
    gb = np.zeros((32, 108), np.float16)
    for r in range(32):
        if r < 2:
            e, l = 12, 8 + r
        else:
            e, l = 13 + (r - 2) // S, (r - 2) % S
        el2 = e - 12
        gb[r, el2 * 27:(el2 + 1) * 27] = g2t16[l]

    # weighted-moment lhsT: wnd[(el2*27+mln), mt*16+e], e = 4mt+el2
    w_flat = np.asarray(w_i, np.float32)[(np.arange(27) // 3) % 3]
    wnd = np.zeros((108, 4, 16), np.float16)
    one8 = np.zeros((108, 4, 2, 16), NPF8)
    for mt in range(4):
        for el2 in range(4):
            e = 4 * mt + el2
            wnd[el2 * 27:(el2 + 1) * 27, mt, e] = \
                (1.0 / w_flat).astype(np.float16)
            one8[el2 * 27:(el2 + 1) * 27, mt, :, e] = 1.0
    wnd = wnd.reshape(108, 64)
    wvec = np.tile(w_flat, 4).reshape(108, 1).astype(np.float32)

    bias_arr = np.asarray(bias, np.float32).reshape(16, 1)

    x = np.asarray(x, np.float32)
    xh_all = x.astype(NPF8)
    xl_all = ((x - xh_all.astype(np.float32)) * 16).astype(NPF8)

    in_maps = []
    for c in range(NCORES):
        b, qq = divmod(c, 4)
        p0 = qq * SLAB
        # windowed views: win[cin, p, y, z, j, k] = x[cin, p0+p, y+j, z+k]
        def _wins(arr):
            sl = arr[b, :, p0:p0 + SLAB_IN]        # (16, 13, 40, 40)
            s0, s1, s2, s3 = sl.strides
            return np.lib.stride_tricks.as_strided(
                sl, (CIN, SLAB_IN, D_OUT, D_OUT, KS, KS),
                (s0, s1, s2, s3, s2, s3))
        xm_arr = np.empty((128, 3, SLAB_IN, D_OUT, D_OUT), NPF8)
        xrun = np.empty((2, 80, SLAB, D_OUT, D_OUT), NPF8)
        for hl, arr in enumerate((xh_all, xl_all)):
            w = _wins(arr)
            # main tiles: row (jkl*16+cin) of tile t = tap jk = t*8+jkl
            wv = w.transpose(4, 5, 0, 1, 2, 3).reshape(
                25, CIN, SLAB_IN, D_OUT, D_OUT)
            tiles = wv[:24].reshape(3, 8, CIN, SLAB_IN, D_OUT, D_OUT) \
                .reshape(3, 128, SLAB_IN, D_OUT, D_OUT) \
                .transpose(1, 0, 2, 3, 4)
            if hl == 0:
                xm_arr[:] = tiles
            else:
                xlm_arr = np.ascontiguousarray(tiles)
            # runt: row (i*16+cin) at out-plane xr = x[cin, xr+i, y+4, z+4]
            rw = w[:, :, :, :, 4, 4]               # (16, 13, 36, 36)
            for i in range(KS):
                xrun[hl, i * CIN:(i + 1) * CIN] = rw[:, i:i + SLAB]
        in_maps.append({
            "xm": np.ascontiguousarray(xm_arr),
            "xlm": xlm_arr,
            "xrh": np.ascontiguousarray(xrun[0]),
            "xrl": np.ascontiguousarray(xrun[1]),
            "wt": wt_arr,
            "wr": wr_arr,
            "ga": np.ascontiguousarray(ga),
            "gb": np.ascontiguousarray(gb),
            "wnd": np.ascontiguousarray(wnd),
            "one8": np.ascontiguousarray(one8),
            "wvec": np.ascontiguousarray(wvec),
            "bias": bias_arr,
        })
    return in_maps


def _run(inputs, trace=False, **run_kwargs):
    inputs = {k: np.asarray(v) for k, v in inputs.items()}
    in_maps = _host_prep(**inputs)
    if "nc" not in _prog_cache:
        _prog_cache["nc"] = _build_program()
    nc = _prog_cache["nc"]
    try:
        res = run_bass_kernel_spmd(nc, in_maps, core_ids=list(range(NCORES)),
                                   trace=trace, **run_kwargs)
    except ModuleNotFoundError as e:
        if "axon_hooks" not in str(e):
            raise
        # Tracing requested (e.g. BASS_TRACE=1) but this axon client has no
        # NTFF profile hook - rerun with tracing disabled.
        os.environ["BASS_NEVER_TRACE"] = "1"
        res = run_bass_kernel_spmd(nc, in_maps, core_ids=list(range(NCORES)),
                                   trace=False, **run_kwargs)
    out = np.empty((B, COUT, D_OUT, D_OUT, D_OUT), np.float32)
    for c in range(NCORES):
        b, qq = divmod(c, 4)
        out[b, :, qq * SLAB:(qq + 1) * SLAB] = res.results[c]["y"]
    return out, res


def kernel(**inputs):
    out, _ = _run(inputs)
    return out
